# revision 30
# baseline (speedup 1.0000x reference)
"""CaptioningRNN forward loss on 8 Trainium2 NeuronCores.

Data-parallel over N: each core runs 16 of the 128 sequences end-to-end;
the scalar loss is all-reduced at the end.

v2 design:
- The RNN scan runs in transposed orientation: state hT [H-part, batch].
  Wh chunks are the PE-stationary operand (bf16 -> fast weight load), the
  16-wide hT block is the moving operand, so each step costs ~64 weight
  loads instead of streaming the full 1024x1024 Wh through the array.
- xW^T for all 32 steps is accumulated directly into all 8 PSUM banks in
  phase 1; scan matmuls accumulate in place (start=False) and tanh reads
  PSUM -> hT in SBUF. No transposes or copies inside the scan.
- Vocab projection (phase 3) uses fp8e4 DoubleRow matmuls (2 contraction
  rows/cycle) with bf16 bias rows; exp/target-extract fused as in v1.

Shapes (hardcoded): N=128, T=33 (32 steps), Dfeat=512, W=512, H=1024,
V=16384.
"""
import numpy as np
import concourse.bass as bass
import concourse.tile as tile
from concourse import bacc, mybir
from concourse.bass_utils import run_bass_kernel_spmd
from concourse.masks import make_identity
from contextlib import ExitStack

dt = mybir.dt
AF = mybir.ActivationFunctionType
OP = mybir.AluOpType
PM = mybir.MatmulPerfMode

N_CORES = 8
NL = 16          # sequences per core
T_STEPS = 32     # scan steps (T-1)
DF = 512         # feature dim
WD = 512         # word vec dim
H = 1024         # hidden dim
V = 16384        # vocab
NTOK = NL * T_STEPS          # 512 tokens per core (t-major: tok = t*16 + n)
NG = NTOK // 128             # 4 groups of 128 tokens
NJ2 = V // 1024              # 16 vocab column tiles (1024 wide, 2 PSUM banks)
OC = H // 128                # 8 hidden-chunk tiles
KC8 = H // 256               # 4 fp8 DoubleRow contraction chunks
KW = WD // 128               # 4 contraction chunks over W
HCOLS = (T_STEPS + 1) * NL   # 528 hT columns (h0 + 32 steps)

_nc_cache = None


def build_program():
    global _nc_cache
    if _nc_cache is not None:
        return _nc_cache
    import os
    klim = os.environ.get("KLIM", "0")  # "12": skip phase3; "3": skip scan
    nc = bacc.Bacc("TRN2", target_bir_lowering=False, debug=False,
                   num_devices=N_CORES)

    # ---- DRAM parameters (per-core shards / replicated weights) ----
    feat_d = nc.dram_tensor("features", [NL, DF], dt.float32, kind="ExternalInput")
    tok_d = nc.dram_tensor("tok", [128, NG], dt.int32, kind="ExternalInput")
    yrel_d = nc.dram_tensor("yrel", [128, NG, NJ2], dt.float32, kind="ExternalInput")
    maskn_d = nc.dram_tensor("maskn", [128, NG], dt.float32, kind="ExternalInput")
    wembed_d = nc.dram_tensor("W_embed", [V, WD], dt.float32, kind="ExternalInput")
    wproj_d = nc.dram_tensor("W_proj", [128, KW, H], dt.bfloat16, kind="ExternalInput")
    bprojT_d = nc.dram_tensor("bprojT", [128, OC], dt.float32, kind="ExternalInput")
    wx_d = nc.dram_tensor("Wx", [128, KW, H], dt.bfloat16, kind="ExternalInput")
    wh_d = nc.dram_tensor("Wh", [128, OC, H], dt.bfloat16, kind="ExternalInput")
    brnn_d = nc.dram_tensor("b_rnn", [1, H], dt.bfloat16, kind="ExternalInput")
    wv_d = nc.dram_tensor("WV8", [NJ2, 128, 8, 2, 512], dt.float8e4,
                          kind="ExternalInput")
    bvoc_d = nc.dram_tensor("b_vocab", [1, NJ2, 2, 1024], dt.float8e4,
                            kind="ExternalInput")
    loss_d = nc.dram_tensor("loss", [1, 4], dt.float32, kind="ExternalOutput")

    with tile.TileContext(nc) as tc, ExitStack() as ctx:
        const = ctx.enter_context(tc.tile_pool(name="const", bufs=1))
        acts = ctx.enter_context(tc.tile_pool(name="acts", bufs=1))
        wts = ctx.enter_context(tc.tile_pool(name="wts", bufs=1))
        scr = ctx.enter_context(tc.tile_pool(name="scr", bufs=2))

        # ---- constants / small inputs ----
        ident128 = const.tile([128, 128], dt.float32)
        make_identity(nc, ident128[:])
        ident16 = const.tile([16, 16], dt.float32)
        make_identity(nc, ident16[:])
        iota_i = const.tile([128, 2, 512], dt.int32)
        nc.gpsimd.iota(iota_i[:], pattern=[[1, 1024]], base=0, channel_multiplier=0)
        iota_f = const.tile([128, 2, 512], dt.float32)
        nc.vector.tensor_copy(iota_f[:], iota_i[:])
        ones_f = const.tile([1, 512], dt.float32)
        nc.vector.memset(ones_f[:], 1.0)
        ones_row = const.tile([1, 512], dt.bfloat16)
        nc.vector.tensor_copy(ones_row[:], ones_f[:])
        ones8 = const.tile([1, 2, 128], dt.float8e4)
        nc.vector.tensor_copy(ones8[:, 0, :], ones_f[:, :128])
        nc.vector.tensor_copy(ones8[:, 1, :], ones_f[:, :128])
        onescol = const.tile([128, 1], dt.float32)
        nc.vector.memset(onescol[:], 1.0)

        tok_t = const.tile([128, NG], dt.int32)
        nc.sync.dma_start(tok_t[:], tok_d.ap())
        yrel_t = const.tile([128, NG, NJ2], dt.float32)
        nc.sync.dma_start(yrel_t[:], yrel_d.ap())
        maskn_t = const.tile([128, NG], dt.float32)
        nc.sync.dma_start(maskn_t[:], maskn_d.ap())
        feat_t = const.tile([NL, DF], dt.float32)
        nc.sync.dma_start(feat_t[:], feat_d.ap())
        bprojT_t = const.tile([128, OC], dt.float32)
        nc.sync.dma_start(bprojT_t[:], bprojT_d.ap())
        brnn_t = const.tile([1, H], dt.bfloat16)
        nc.sync.dma_start(brnn_t[:], brnn_d.ap())

        # ---- persistent activations ----
        hT_all = acts.tile([128, OC, HCOLS], dt.bfloat16)   # h transposed, bf16
        hT8_all = acts.tile([128, OC, HCOLS], dt.float8e4)  # fp8 copy for phase 3
        s_cols = acts.tile([128, NG, NJ2], dt.float32)      # exp-sum partials
        t_cols = acts.tile([128, NG, NJ2], dt.float32)      # target partials

        # ---- phase 1a: embed gather + xT (cast bf16) + h0T ----
        xT_all = acts.tile([128, KW, NTOK], dt.bfloat16)
        with ExitStack() as ectx:
            early = ectx.enter_context(tc.tile_pool(name="early", bufs=1))
            psE = ectx.enter_context(tc.tile_pool(name="psE", bufs=2, space="PSUM"))
            wp_t = early.tile([128, KW, H], dt.bfloat16)
            nc.sync.dma_start(wp_t[:], wproj_d.ap())
            x_all = early.tile([128, NG, WD], dt.float32)
            for g in range(NG):
                nc.gpsimd.indirect_dma_start(
                    out=x_all[:, g, :], out_offset=None,
                    in_=wembed_d.ap(),
                    in_offset=bass.IndirectOffsetOnAxis(ap=tok_t[:, g:g + 1], axis=0),
                )
            for g in range(NG):
                ps_x = psE.tile([128, KW, 128], dt.float32, space="PSUM", tag="tr")
                for wc in range(KW):
                    nc.tensor.transpose(out=ps_x[:, wc, :],
                                        in_=x_all[:, g, wc * 128:(wc + 1) * 128],
                                        identity=ident128[:])
                nc.vector.tensor_copy(xT_all[:, :, g * 128:(g + 1) * 128], ps_x[:])

            # featT (bf16) then h0T = (features @ W_proj + b_proj)^T
            ps_f = psE.tile([128, KW, 16], dt.float32, space="PSUM", tag="tr2")
            for kc in range(KW):
                nc.tensor.transpose(out=ps_f[:, kc, :],
                                    in_=feat_t[:, kc * 128:(kc + 1) * 128],
                                    identity=ident16[:])
            featT = early.tile([128, KW, 16], dt.bfloat16)
            nc.vector.tensor_copy(featT[:], ps_f[:])
            for oc in range(OC):
                ps_h = psE.tile([128, 16], dt.float32, space="PSUM", tag="h0")
                for kc in range(KW):
                    nc.tensor.matmul(
                        out=ps_h[:],
                        lhsT=wp_t[:, kc, oc * 128:(oc + 1) * 128],
                        rhs=featT[:, kc, :],
                        start=(kc == 0), stop=(kc == KW - 1))
                nc.scalar.add(hT_all[:, oc, 0:NL], ps_h[:], bprojT_t[:, oc:oc + 1])
            nc.vector.tensor_copy(hT8_all[:, :, 0:NL], hT_all[:, :, 0:NL])

        # ---- phase 1b: xW^T + b_rnn into all 8 PSUM banks ----
        wx_t = wts.tile([128, KW, H], dt.bfloat16)
        nc.sync.dma_start(wx_t[:], wx_d.ap())
        wh_t = wts.tile([128, OC, H], dt.bfloat16)
        nc.sync.dma_start(wh_t[:], wh_d.ap())

        with ExitStack() as sctx:
            psS = sctx.enter_context(tc.tile_pool(name="psS", bufs=1, space="PSUM"))
            ps_all = psS.tile([128, OC, NTOK], dt.float32, space="PSUM")
            for oc in range(OC):
                for kc in range(KW):
                    nc.tensor.matmul(
                        out=ps_all[:, oc, :],
                        lhsT=wx_t[:, kc, oc * 128:(oc + 1) * 128],
                        rhs=xT_all[:, kc, :],
                        start=(kc == 0), stop=False)
                nc.tensor.matmul(
                    out=ps_all[:, oc, :],
                    lhsT=brnn_t[:, oc * 128:(oc + 1) * 128],
                    rhs=ones_row[:],
                    start=False, stop=True)

            # ---- phase 2: the scan ----
            if klim == "3":
                nc.vector.memset(hT8_all[:].bitcast(dt.float32), 0.001)
            for b in range(T_STEPS if klim != "3" else 0):
                lo, hi = b * NL, (b + 1) * NL
                for oc in range(OC):
                    for kc in range(OC):
                        nc.tensor.matmul(
                            out=ps_all[:, oc, lo:hi],
                            lhsT=wh_t[:, kc, oc * 128:(oc + 1) * 128],
                            rhs=hT_all[:, kc, lo:hi],
                            start=False, stop=(kc == OC - 1),
                            skip_group_check=True)
                for half in range(2):
                    o0, o1 = half * 4, (half + 1) * 4
                    nc.scalar.activation(
                        hT_all[:, o0:o1, hi:hi + NL],
                        ps_all[:, o0:o1, lo:hi], AF.Tanh)
                nc.vector.tensor_copy(hT8_all[:, :, hi:hi + NL],
                                      hT_all[:, :, hi:hi + NL])

        # ---- phase 3: scores (fp8 DoubleRow) + fused CE pieces ----
        with ExitStack() as pctx:
            wvp = pctx.enter_context(tc.tile_pool(name="wvp", bufs=4))
            psB = pctx.enter_context(tc.tile_pool(name="psB", bufs=3, space="PSUM"))
            psL = pctx.enter_context(tc.tile_pool(name="psL", bufs=1, space="PSUM"))
            bvoc_t = acts.tile([1, NJ2, 2, 1024], dt.float8e4)
            nc.sync.dma_start(bvoc_t[:], bvoc_d.ap())
            if klim == "12":
                nc.vector.memset(s_cols[:], 1.0)
                nc.vector.memset(t_cols[:], 0.0)
            for jj in range(NJ2 if klim != "12" else 0):
                wv_t = wvp.tile([128, 8, 2, 512], dt.float8e4, tag="wv")
                nc.sync.dma_start(wv_t[:], wv_d.ap()[jj])
                for m in range(NG):
                    t0 = NL + m * 128
                    ps = psB.tile([128, 2, 512], dt.float32, space="PSUM", tag="big")
                    for half in range(2):
                        for kc in range(KC8):
                            nc.tensor.matmul(
                                out=ps[:, half, :],
                                lhsT=hT8_all[:, 2 * kc:2 * kc + 2, t0:t0 + 128],
                                rhs=wv_t[:, half * KC8 + kc, :, :],
                                start=(kc == 0), stop=False,
                                perf_mode=PM.DoubleRow)
                        nc.tensor.matmul(
                            out=ps[:, half, :], lhsT=ones8[:],
                            rhs=bvoc_t[:, jj, :, half * 512:(half + 1) * 512],
                            start=False, stop=True,
                            perf_mode=PM.DoubleRow)
                    exp_s = scr.tile([128, 2, 512], dt.float32, tag="exp")
                    nc.scalar.activation(exp_s[:], ps[:], AF.Exp,
                                         accum_out=s_cols[:, m, jj:jj + 1])
                    stt_s = scr.tile([128, 2, 512], dt.float32, tag="stt")
                    nc.vector.scalar_tensor_tensor(
                        out=stt_s[:], in0=iota_f[:], scalar=yrel_t[:, m, jj:jj + 1],
                        in1=ps[:], op0=OP.is_equal, op1=OP.mult,
                        accum_out=t_cols[:, m, jj:jj + 1])

            # ---- phase 4: loss ----
            s_red = acts.tile([128, NG], dt.float32)
            t_red = acts.tile([128, NG], dt.float32)
            for m in range(NG):
                nc.vector.tensor_reduce(out=s_red[:, m:m + 1], in_=s_cols[:, m, :],
                                        axis=mybir.AxisListType.X, op=OP.add)
                nc.vector.tensor_reduce(out=t_red[:, m:m + 1], in_=t_cols[:, m, :],
                                        axis=mybir.AxisListType.X, op=OP.add)
            ln_s = acts.tile([128, NG], dt.float32)
            nc.scalar.activation(ln_s[:], s_red[:], AF.Ln)
            diff = acts.tile([128, NG], dt.float32)
            nc.vector.tensor_tensor(out=diff[:], in0=ln_s[:], in1=t_red[:],
                                    op=OP.subtract)
            masked = acts.tile([128, NG], dt.float32)
            nc.vector.tensor_tensor(out=masked[:], in0=diff[:], in1=maskn_t[:],
                                    op=OP.mult)
            # per-core partial loss only; the cross-core sum (the "unshard"
            # of a sum-reduced output) happens on the host
            ps_l = psL.tile([1, NG], dt.float32, space="PSUM", tag="red")
            nc.tensor.matmul(out=ps_l[:], lhsT=onescol[:], rhs=masked[:],
                             start=True, stop=True)
            lsb = acts.tile([1, 4], dt.float32)
            nc.vector.tensor_copy(lsb[:], ps_l[:])
            lfin = acts.tile([1, 4], dt.float32)
            nc.vector.memset(lfin[:], 0.0)
            nc.vector.tensor_reduce(out=lfin[:, :1], in_=lsb[:],
                                    axis=mybir.AxisListType.X, op=OP.add)
            nc.sync.dma_start(loss_d.ap(), lfin[:])

    nc.compile()
    _nc_cache = nc
    return nc


def make_in_maps(features, captions, W_proj, b_proj, W_embed, Wx, Wh, b,
                 W_vocab, b_vocab):
    bf16 = dt.np(dt.bfloat16)
    f8 = dt.np(dt.float8e4)
    features = np.asarray(features, dtype=np.float32)
    cap = np.asarray(captions).astype(np.int64)
    # Wv [H, V] -> [NJ2, 128, (half,kc), 2, 512]: DR pair (p, r) <-> H row
    # (2kc+r)*128+p; vocab tiled as 16 x (2 halves x 512); one DMA per jj
    wv8 = (np.asarray(W_vocab, dtype=np.float32)
           .reshape(KC8, 2, 128, NJ2, 2, 512).transpose(3, 2, 4, 0, 1, 5)
           .reshape(NJ2, 128, 8, 2, 512).astype(f8))
    bv8 = np.zeros((1, NJ2, 2, 1024), dtype=np.float32)
    bv8[0, :, 0, :] = np.asarray(b_vocab, dtype=np.float32).reshape(NJ2, 1024)
    bprojT = np.asarray(b_proj, dtype=np.float32).reshape(OC, 128).T
    shared = {
        "W_embed": np.asarray(W_embed, dtype=np.float32),
        "W_proj": np.ascontiguousarray(
            np.asarray(W_proj, np.float32).reshape(KW, 128, H).transpose(1, 0, 2)
        ).astype(bf16),
        "bprojT": np.ascontiguousarray(bprojT),
        "Wx": np.ascontiguousarray(
            np.asarray(Wx, np.float32).reshape(KW, 128, H).transpose(1, 0, 2)
        ).astype(bf16),
        "Wh": np.ascontiguousarray(
            np.asarray(Wh, np.float32).reshape(OC, 128, H).transpose(1, 0, 2)
        ).astype(bf16),
        "b_rnn": np.asarray(b, dtype=np.float32).reshape(1, H).astype(bf16),
        "WV8": np.ascontiguousarray(wv8),
        "b_vocab": bv8.astype(f8),
    }
    in_maps = []
    for c in range(N_CORES):
        capc = cap[c * NL:(c + 1) * NL]              # (16, 33)
        tok_tm = capc[:, :T_STEPS].T.reshape(NTOK)   # token ids, t-major
        y_tm = capc[:, 1:].T.reshape(NTOK)           # targets, t-major
        tok_pg = tok_tm.reshape(NG, 128).T.astype(np.int32).copy()   # (128, NG)
        y_pg = y_tm.reshape(NG, 128).T                               # (128, NG)
        yrel = (y_pg[:, :, None].astype(np.float32)
                - (np.arange(NJ2, dtype=np.float32) * 1024)[None, None, :])
        maskn = (y_pg != 0).astype(np.float32) / 128.0
        in_maps.append({
            "features": features[c * NL:(c + 1) * NL],
            "tok": tok_pg,
            "yrel": np.ascontiguousarray(yrel),
            "maskn": np.ascontiguousarray(maskn),
            **shared,
        })
    return in_maps


def kernel(**inputs) -> np.ndarray:
    nc = build_program()
    in_maps = make_in_maps(**inputs)
    res = run_bass_kernel_spmd(nc, in_maps, list(range(N_CORES)))
    return np.float32(sum(res.results[c]["loss"][0, 0] for c in range(N_CORES)))


# revision 46
# speedup vs baseline: 1.2536x; 1.2536x over previous
"""CaptioningRNN forward loss on 8 Trainium2 NeuronCores.

Data-parallel over N: each core runs 16 of the 128 sequences end-to-end;
the scalar loss is all-reduced at the end.

v2 design:
- The RNN scan runs in transposed orientation: state hT [H-part, batch].
  Wh chunks are the PE-stationary operand (bf16 -> fast weight load), the
  16-wide hT block is the moving operand, so each step costs ~64 weight
  loads instead of streaming the full 1024x1024 Wh through the array.
- xW^T for all 32 steps is accumulated directly into all 8 PSUM banks in
  phase 1; scan matmuls accumulate in place (start=False) and tanh reads
  PSUM -> hT in SBUF. No transposes or copies inside the scan.
- Vocab projection (phase 3) uses fp8e4 DoubleRow matmuls (2 contraction
  rows/cycle) with bf16 bias rows; exp/target-extract fused as in v1.

Shapes (hardcoded): N=128, T=33 (32 steps), Dfeat=512, W=512, H=1024,
V=16384.
"""
import numpy as np
import concourse.bass as bass
import concourse.tile as tile
from concourse import bacc, mybir
from concourse.bass_utils import run_bass_kernel_spmd
from concourse.masks import make_identity
from contextlib import ExitStack

dt = mybir.dt
AF = mybir.ActivationFunctionType
OP = mybir.AluOpType
PM = mybir.MatmulPerfMode

N_CORES = 8
NL = 16          # sequences per core
T_STEPS = 32     # scan steps (T-1)
DF = 512         # feature dim
WD = 512         # word vec dim
H = 1024         # hidden dim
V = 16384        # vocab
NTOK = NL * T_STEPS          # 512 tokens per core (t-major: tok = t*16 + n)
NG = NTOK // 128             # 4 groups of 128 tokens
NJ2 = V // 1024              # 16 vocab column tiles (1024 wide, 2 PSUM banks)
OC = H // 128                # 8 hidden-chunk tiles
KC8 = H // 256               # 4 fp8 DoubleRow contraction chunks
KW = WD // 128               # 4 contraction chunks over W
HCOLS = (T_STEPS + 1) * NL   # 528 hT columns (h0 + 32 steps)

_nc_cache = {}


def build_program(nobias=False):
    import os
    klim = os.environ.get("KLIM", "0")  # "12": skip phase3; "3": skip scan
    kloop = int(os.environ.get("KLOOP", "1"))  # HW-loop reps for timing
    key = (klim, kloop, nobias)
    if key in _nc_cache:
        return _nc_cache[key]
    nc = bacc.Bacc("TRN2", target_bir_lowering=False, debug=False,
                   num_devices=N_CORES)

    # ---- DRAM parameters (per-core shards / replicated weights) ----
    feat_d = nc.dram_tensor("features", [NL, DF], dt.float32, kind="ExternalInput")
    tok_d = nc.dram_tensor("tok", [128, NG], dt.int32, kind="ExternalInput")
    yrel_d = nc.dram_tensor("yrel", [128, NG, NJ2], dt.float32, kind="ExternalInput")
    maskn_d = nc.dram_tensor("maskn", [128, NG], dt.float32, kind="ExternalInput")
    wembed_d = nc.dram_tensor("W_embed", [V, WD], dt.float32, kind="ExternalInput")
    wproj_d = nc.dram_tensor("W_proj", [128, KW, H], dt.bfloat16, kind="ExternalInput")
    wx_d = nc.dram_tensor("Wx", [128, KW, H], dt.bfloat16, kind="ExternalInput")
    wh_d = nc.dram_tensor("Wh", [128, OC, H], dt.bfloat16, kind="ExternalInput")
    if not nobias:
        bprojT_d = nc.dram_tensor("bprojT", [128, OC], dt.float32,
                                  kind="ExternalInput")
        brnn_d = nc.dram_tensor("b_rnn", [1, H], dt.bfloat16, kind="ExternalInput")
        bvoc_d = nc.dram_tensor("b_vocab", [1, NJ2, 2, 1024], dt.float8e4,
                                kind="ExternalInput")
    wv_d = nc.dram_tensor("WV8", [NJ2, 128, 8, 2, 512], dt.float8e4,
                          kind="ExternalInput")
    loss_d = nc.dram_tensor("loss", [1, 4], dt.float32, kind="ExternalOutput")

    with tile.TileContext(nc) as tc, ExitStack() as ctx:
        if kloop > 1:
            ctx.enter_context(tc.For_i(0, kloop, 1))
        const = ctx.enter_context(tc.tile_pool(name="const", bufs=1))
        acts = ctx.enter_context(tc.tile_pool(name="acts", bufs=1))
        wts = ctx.enter_context(tc.tile_pool(name="wts", bufs=1))
        scr = ctx.enter_context(tc.tile_pool(name="scr", bufs=2))

        # ---- constants / small inputs ----
        ident128 = const.tile([128, 128], dt.float32)
        make_identity(nc, ident128[:])
        ident128b = const.tile([128, 128], dt.bfloat16)
        nc.vector.tensor_copy(ident128b[:], ident128[:])
        ident16 = const.tile([16, 16], dt.float32)
        make_identity(nc, ident16[:])
        iota_i = const.tile([128, 2, 512], dt.int32)
        nc.gpsimd.iota(iota_i[:], pattern=[[1, 1024]], base=0, channel_multiplier=0)
        iota_f = const.tile([128, 2, 512], dt.float32)
        nc.vector.tensor_copy(iota_f[:], iota_i[:])
        ones_f = const.tile([1, 512], dt.float32)
        nc.vector.memset(ones_f[:], 1.0)
        ones_row = const.tile([1, 512], dt.bfloat16)
        nc.vector.tensor_copy(ones_row[:], ones_f[:])
        ones8 = const.tile([1, 2, 128], dt.float8e4)
        nc.vector.tensor_copy(ones8[:, 0, :], ones_f[:, :128])
        nc.vector.tensor_copy(ones8[:, 1, :], ones_f[:, :128])
        onescol = const.tile([128, 1], dt.float32)
        nc.vector.memset(onescol[:], 1.0)

        tok_t = const.tile([128, NG], dt.int32)
        nc.sync.dma_start(tok_t[:], tok_d.ap())
        yrel_t = const.tile([128, NG, NJ2], dt.float32)
        nc.sync.dma_start(yrel_t[:], yrel_d.ap())
        maskn_t = const.tile([128, NG], dt.float32)
        nc.sync.dma_start(maskn_t[:], maskn_d.ap())
        feat_t = const.tile([NL, DF], dt.float32)
        nc.sync.dma_start(feat_t[:], feat_d.ap())
        if not nobias:
            bprojT_t = const.tile([128, OC], dt.float32)
            nc.sync.dma_start(bprojT_t[:], bprojT_d.ap())
            brnn_t = const.tile([1, H], dt.bfloat16)
            nc.sync.dma_start(brnn_t[:], brnn_d.ap())

        # ---- persistent activations ----
        hT_all = acts.tile([128, OC, HCOLS], dt.bfloat16)   # h transposed, bf16
        hT8_all = acts.tile([128, OC, HCOLS], dt.float8e4)  # fp8 copy for phase 3
        s_cols = acts.tile([128, NG, NJ2], dt.float32)      # exp-sum partials
        t_cols = acts.tile([128, NG, NJ2], dt.float32)      # target partials

        # ---- phase 1a: embed gather + xT (cast bf16) + h0T ----
        xT_all = acts.tile([128, KW, NTOK], dt.bfloat16)
        with ExitStack() as ectx:
            early = ectx.enter_context(tc.tile_pool(name="early", bufs=1))
            psE = ectx.enter_context(tc.tile_pool(name="psE", bufs=2, space="PSUM"))
            wp_t = early.tile([128, KW, H], dt.bfloat16)
            nc.sync.dma_start(wp_t[:], wproj_d.ap())
            x_all = early.tile([128, NG, WD], dt.float32)
            for g in range(NG):
                nc.gpsimd.indirect_dma_start(
                    out=x_all[:, g, :], out_offset=None,
                    in_=wembed_d.ap(),
                    in_offset=bass.IndirectOffsetOnAxis(ap=tok_t[:, g:g + 1], axis=0),
                )
            for g in range(NG):
                ps_x = psE.tile([128, KW, 128], dt.float32, space="PSUM", tag="tr")
                for wc in range(KW):
                    nc.tensor.transpose(out=ps_x[:, wc, :],
                                        in_=x_all[:, g, wc * 128:(wc + 1) * 128],
                                        identity=ident128[:])
                nc.vector.tensor_copy(xT_all[:, :, g * 128:(g + 1) * 128], ps_x[:])

            # featT (bf16) then h0T = (features @ W_proj + b_proj)^T
            ps_f = psE.tile([128, KW, 16], dt.float32, space="PSUM", tag="tr2")
            for kc in range(KW):
                nc.tensor.transpose(out=ps_f[:, kc, :],
                                    in_=feat_t[:, kc * 128:(kc + 1) * 128],
                                    identity=ident16[:])
            featT = early.tile([128, KW, 16], dt.bfloat16)
            nc.vector.tensor_copy(featT[:], ps_f[:])
            for oc in range(OC):
                ps_h = psE.tile([128, 16], dt.float32, space="PSUM", tag="h0")
                for kc in range(KW):
                    nc.tensor.matmul(
                        out=ps_h[:],
                        lhsT=wp_t[:, kc, oc * 128:(oc + 1) * 128],
                        rhs=featT[:, kc, :],
                        start=(kc == 0), stop=(kc == KW - 1))
                if nobias:
                    nc.scalar.copy(hT_all[:, oc, 0:NL], ps_h[:])
                else:
                    nc.scalar.add(hT_all[:, oc, 0:NL], ps_h[:],
                                  bprojT_t[:, oc:oc + 1])
            nc.vector.tensor_copy(hT8_all[:, :, 0:NL], hT_all[:, :, 0:NL])

        # ---- phase 1b: xW^T (+ b_rnn) into SBUF ----
        wx_t = wts.tile([128, KW, H], dt.bfloat16)
        nc.sync.dma_start(wx_t[:], wx_d.ap())
        wh_t = wts.tile([128, OC, H], dt.bfloat16)
        nc.sync.dma_start(wh_t[:], wh_d.ap())

        xw_sb = acts.tile([128, OC, NTOK], dt.bfloat16)
        with ExitStack() as xctx:
            psX = xctx.enter_context(tc.tile_pool(name="psX", bufs=2, space="PSUM"))
            for oc in range(OC):
                ps_xw = psX.tile([128, NTOK], dt.float32, space="PSUM", tag="xw")
                for kc in range(KW):
                    nc.tensor.matmul(
                        out=ps_xw[:],
                        lhsT=wx_t[:, kc, oc * 128:(oc + 1) * 128],
                        rhs=xT_all[:, kc, :],
                        start=(kc == 0), stop=(nobias and kc == KW - 1))
                if not nobias:
                    nc.tensor.matmul(
                        out=ps_xw[:],
                        lhsT=brnn_t[:, oc * 128:(oc + 1) * 128],
                        rhs=ones_row[:],
                        start=False, stop=True)
                nc.vector.tensor_copy(xw_sb[:, oc, :], ps_xw[:])

        # ---- phases 2+3, interleaved: scan steps with vocab groups ----
        with ExitStack() as pctx:
            wvp = pctx.enter_context(tc.tile_pool(name="wvp", bufs=5))
            psSc = pctx.enter_context(tc.tile_pool(name="psSc", bufs=2, space="PSUM"))
            psB = pctx.enter_context(tc.tile_pool(name="psB", bufs=3, space="PSUM"))
            if not nobias:
                bvoc_t = acts.tile([1, NJ2, 2, 1024], dt.float8e4)
                nc.sync.dma_start(bvoc_t[:], bvoc_d.ap())

            def emit_step(b):
                lo, hi = b * NL, (b + 1) * NL
                ps_step = psSc.tile([128, OC, NL], dt.float32, space="PSUM",
                                    tag="scan")
                for oc in range(OC):
                    nc.tensor.matmul(out=ps_step[:, oc, :], lhsT=ident128b[:],
                                     rhs=xw_sb[:, oc, lo:hi],
                                     start=True, stop=False)
                    for kc in range(OC):
                        nc.tensor.matmul(
                            out=ps_step[:, oc, :],
                            lhsT=wh_t[:, kc, oc * 128:(oc + 1) * 128],
                            rhs=hT_all[:, kc, lo:hi],
                            start=False, stop=(kc == OC - 1))
                for half in range(2):
                    o0, o1 = half * 4, (half + 1) * 4
                    nc.scalar.activation(hT_all[:, o0:o1, hi:hi + NL],
                                         ps_step[:, o0:o1, :], AF.Tanh)
                nc.vector.tensor_copy(hT8_all[:, :, hi:hi + NL],
                                      hT_all[:, :, hi:hi + NL])

            def load_wv(jj):
                wv_t = wvp.tile([128, 8, 2, 512], dt.float8e4, tag="wv")
                nc.sync.dma_start(wv_t[:], wv_d.ap()[jj])
                return wv_t

            def emit_group(jj, m, wv_t):
                t0 = NL + m * 128
                ps = psB.tile([128, 2, 512], dt.float32, space="PSUM", tag="big")
                for half in range(2):
                    for kc in range(KC8):
                        nc.tensor.matmul(
                            out=ps[:, half, :],
                            lhsT=hT8_all[:, 2 * kc:2 * kc + 2, t0:t0 + 128],
                            rhs=wv_t[:, half * KC8 + kc, :, :],
                            start=(kc == 0),
                            stop=(nobias and kc == KC8 - 1),
                            perf_mode=PM.DoubleRow)
                    if not nobias:
                        nc.tensor.matmul(
                            out=ps[:, half, :], lhsT=ones8[:],
                            rhs=bvoc_t[:, jj, :, half * 512:(half + 1) * 512],
                            start=False, stop=True,
                            perf_mode=PM.DoubleRow)
                exp_s = scr.tile([128, 2, 512], dt.float32, tag="exp")
                nc.scalar.activation(exp_s[:], ps[:], AF.Exp,
                                     accum_out=s_cols[:, m, jj:jj + 1])
                stt_s = scr.tile([128, 2, 512], dt.float32, tag="stt")
                nc.vector.scalar_tensor_tensor(
                    out=stt_s[:], in0=iota_f[:], scalar=yrel_t[:, m, jj:jj + 1],
                    in1=ps[:], op0=OP.is_equal, op1=OP.mult,
                    accum_out=t_cols[:, m, jj:jj + 1])

            do_p3 = klim != "12"
            do_scan = klim != "3"
            if not do_scan:
                nc.vector.memset(hT8_all[:].bitcast(dt.float32), 0.001)
            if not do_p3:
                nc.vector.memset(s_cols[:], 1.0)
                nc.vector.memset(t_cols[:], 0.0)
            LEAD = 4          # prefetch wv this many steps before first use
            wv_tiles = {}
            for b in range(T_STEPS if do_scan else 0):
                if do_p3:
                    jpre = b - (16 - LEAD)
                    if 0 <= jpre < NJ2:
                        wv_tiles[jpre] = load_wv(jpre)
                emit_step(b)
                if do_p3:
                    juse = b - 16
                    if 0 <= juse < NJ2:
                        emit_group(juse, 0, wv_tiles[juse])
                        emit_group(juse, 1, wv_tiles.pop(juse))
            if do_p3:
                if not do_scan:
                    for jj in range(NJ2):
                        wv_t = load_wv(jj)
                        emit_group(jj, 0, wv_t)
                        emit_group(jj, 1, wv_t)
                for jj in range(NJ2):
                    wv_t = load_wv(jj)
                    emit_group(jj, 2, wv_t)
                    emit_group(jj, 3, wv_t)

            # ---- phase 4: loss ----
            s_red = acts.tile([128, NG], dt.float32)
            t_red = acts.tile([128, NG], dt.float32)
            for m in range(NG):
                nc.vector.tensor_reduce(out=s_red[:, m:m + 1], in_=s_cols[:, m, :],
                                        axis=mybir.AxisListType.X, op=OP.add)
                nc.vector.tensor_reduce(out=t_red[:, m:m + 1], in_=t_cols[:, m, :],
                                        axis=mybir.AxisListType.X, op=OP.add)
            ln_s = acts.tile([128, NG], dt.float32)
            nc.scalar.activation(ln_s[:], s_red[:], AF.Ln)
            diff = acts.tile([128, NG], dt.float32)
            nc.vector.tensor_tensor(out=diff[:], in0=ln_s[:], in1=t_red[:],
                                    op=OP.subtract)
            masked = acts.tile([128, NG], dt.float32)
            nc.vector.tensor_tensor(out=masked[:], in0=diff[:], in1=maskn_t[:],
                                    op=OP.mult)
            # per-core partial loss only; the cross-core sum (the "unshard"
            # of a sum-reduced output) happens on the host
            ps_l = psB.tile([128, 2, 512], dt.float32, space="PSUM", tag="big")
            nc.tensor.matmul(out=ps_l[:1, 0, :NG], lhsT=onescol[:], rhs=masked[:],
                             start=True, stop=True)
            lsb = acts.tile([1, 4], dt.float32)
            nc.vector.tensor_copy(lsb[:], ps_l[:1, 0, :NG])
            lfin = acts.tile([1, 4], dt.float32)
            nc.vector.memset(lfin[:], 0.0)
            nc.vector.tensor_reduce(out=lfin[:, :1], in_=lsb[:],
                                    axis=mybir.AxisListType.X, op=OP.add)
            nc.sync.dma_start(loss_d.ap(), lfin[:])

    nc.compile()
    _nc_cache[key] = nc
    return nc


def make_in_maps(features, captions, W_proj, b_proj, W_embed, Wx, Wh, b,
                 W_vocab, b_vocab, nobias=False):
    bf16 = dt.np(dt.bfloat16)
    f8 = dt.np(dt.float8e4)
    features = np.asarray(features, dtype=np.float32)
    cap = np.asarray(captions).astype(np.int64)
    # Wv [H, V] -> [NJ2, 128, (half,kc), 2, 512]: DR pair (p, r) <-> H row
    # (2kc+r)*128+p; vocab tiled as 16 x (2 halves x 512); one DMA per jj
    wv8 = (np.asarray(W_vocab, dtype=np.float32)
           .reshape(KC8, 2, 128, NJ2, 2, 512).transpose(3, 2, 4, 0, 1, 5)
           .reshape(NJ2, 128, 8, 2, 512).astype(f8))
    bv8 = np.zeros((1, NJ2, 2, 1024), dtype=np.float32)
    bv8[0, :, 0, :] = np.asarray(b_vocab, dtype=np.float32).reshape(NJ2, 1024)
    bprojT = np.asarray(b_proj, dtype=np.float32).reshape(OC, 128).T
    shared = {
        "W_embed": np.asarray(W_embed, dtype=np.float32),
        "W_proj": np.ascontiguousarray(
            np.asarray(W_proj, np.float32).reshape(KW, 128, H).transpose(1, 0, 2)
        ).astype(bf16),
        "bprojT": np.ascontiguousarray(bprojT),
        "Wx": np.ascontiguousarray(
            np.asarray(Wx, np.float32).reshape(KW, 128, H).transpose(1, 0, 2)
        ).astype(bf16),
        "Wh": np.ascontiguousarray(
            np.asarray(Wh, np.float32).reshape(OC, 128, H).transpose(1, 0, 2)
        ).astype(bf16),
        "b_rnn": np.asarray(b, dtype=np.float32).reshape(1, H).astype(bf16),
        "WV8": np.ascontiguousarray(wv8),
        "b_vocab": bv8.astype(f8),
    }
    if nobias:
        for k in ("bprojT", "b_rnn", "b_vocab"):
            shared.pop(k)
    in_maps = []
    for c in range(N_CORES):
        capc = cap[c * NL:(c + 1) * NL]              # (16, 33)
        tok_tm = capc[:, :T_STEPS].T.reshape(NTOK)   # token ids, t-major
        y_tm = capc[:, 1:].T.reshape(NTOK)           # targets, t-major
        tok_pg = tok_tm.reshape(NG, 128).T.astype(np.int32).copy()   # (128, NG)
        y_pg = y_tm.reshape(NG, 128).T                               # (128, NG)
        yrel = (y_pg[:, :, None].astype(np.float32)
                - (np.arange(NJ2, dtype=np.float32) * 1024)[None, None, :])
        maskn = (y_pg != 0).astype(np.float32) / 128.0
        in_maps.append({
            "features": features[c * NL:(c + 1) * NL],
            "tok": tok_pg,
            "yrel": np.ascontiguousarray(yrel),
            "maskn": np.ascontiguousarray(maskn),
            **shared,
        })
    return in_maps


def prepare(inputs):
    nobias = (not np.any(np.asarray(inputs["b_vocab"]))
              and not np.any(np.asarray(inputs["b"]))
              and not np.any(np.asarray(inputs["b_proj"])))
    nc = build_program(nobias=nobias)
    in_maps = make_in_maps(**inputs, nobias=nobias)
    return nc, in_maps


def kernel(**inputs) -> np.ndarray:
    nc, in_maps = prepare(inputs)
    res = run_bass_kernel_spmd(nc, in_maps, list(range(N_CORES)))
    return np.float32(sum(res.results[c]["loss"][0, 0] for c in range(N_CORES)))
